# revision 33
# baseline (speedup 1.0000x reference)
"""Trainium2 Bass kernel for the ergodicity loss.

Math: for x[T=512, B=16, N=32, d=2] in [0,1]^2 and modes (k0,k1) in {0..9}^2:
    basis = cos(pi*k0*x0) * cos(pi*k1*x1)                    (separable!)
    coeffs[b, k0, k1] = sum_{t,n} basis / (T*N) / nf[k1]
    loss = mean((nw * (coeffs - cd))**2)

Device strategy (8 cores, data-parallel over T: 64 timesteps/core):
  - Per core, per batch: 2048 points = 16 chunks x 128 partitions.
  - Both coordinate dims processed as ONE fused elementwise stream, in two
    fg-halves h for pipelining (xx[p, h*256 + dd*128 + fg_l*8 + b']):
      w = xx*(k/2) + 128.75     (DVE fused tensor_scalar per (k, h), 2x mode)
        w in [128.75, 133.25]: exponent pinned to 7, so the mantissa holds
        (v + 0.5)*2^16 in fixed point, where v = x*k/2 + 0.25 and
        sin(2*pi*v) == cos(pi*k*x).
      s = low int16 half of w    (no instruction at all -- the ACT input AP
        reads the low 16-bit lane of each fp32 at stride 2; the signed
        16-bit wraparound IS the mod-2^16 range reduction, absorbed by sin's
        periodicity up to a sign flip)
      c = Sin(s * (-2pi/2^16))   (ACT; arg in (-pi, pi] == Sin's valid table
        range; == -sin(2*pi*(v mod 1 -+ 0.5)) == sin(2*pi*v) == cos(pi*k*x)),
        written as bf16.
    Phase quantization at 2^-16 (~1e-4 rad) is far below the bf16 rounding
    of the cos values themselves; measured loss error vs reference ~7e-6.
  - coeffs partial sums on PE (bf16 inputs, fp32 PSUM): per chunk fg,
    lhsT = C0[:, fg*80:+80] (cols (k0, b')), rhs = C1 same -> PSUM[80, 80]
    accumulated over the 16 f-chunks; 2 batch-groups. Off-diagonal batch
    blocks are garbage, ignored at gather.
  - Output per core: sout[80, 160] raw PSUM dump (2 groups side by side).
Host: sum the 8 per-core partials, extract diagonal blocks, apply the tiny
[16, 100] normalization + weighted MSE.
"""
import numpy as np

T, B, NA, D = 512, 16, 32, 2
KMAX = 10
NCORES = 8
TLOC = T // NCORES          # 64 timesteps per core
NF = 16                     # point-chunks per batch (64*32/128)
KN = KMAX * KMAX

_STATE = {}

# tuning knobs: ts_pool_ks = k-values whose tensor_scalar runs on the (idle)
# Pool engine instead of DVE; dma_split = issue the h1 input DMA from the
# gpsimd engine so the two half-loads ride different DMA queues
CFG = {"ts_pool_ks": (), "dma_split": False, "and_free": True}


def _np_constants():
    """Replicates reference._constants() exactly in numpy (L = ones)."""
    L = np.ones(D, dtype=np.float32)
    grids = np.meshgrid(*[np.arange(KMAX) for _ in range(D)], indexing="ij")
    K = np.stack(grids, -1).reshape(-1, D).astype(np.float32)          # [100, 2]
    k_scaled = K * np.pi / L
    nf = np.where(K[:, -1] != 0, np.sqrt(L[-1] / 2.0), 1.0).astype(np.float32)
    nw = ((1.0 + (k_scaled ** 2).sum(-1)) ** (-(D + 1) / 2.0) * 100.0).astype(np.float32)
    safe_k = np.where(K != 0, k_scaled, 1.0)
    term = np.where(K != 0,
                    (np.exp(1j * k_scaled * L) - 1.0) / (1j * safe_k * L),
                    1.0 + 0j)
    cd = (term.prod(-1).real / nf).astype(np.float32)                  # [100]
    return nf, nw, cd


def _build(reps: int = 1, loop: bool = False, cfg: dict | None = None):
    import concourse.tile as tile
    from concourse import bacc, mybir

    cfg = {**CFG, **(cfg or {})}
    f32 = mybir.dt.float32
    i32 = mybir.dt.int32
    bf16 = mybir.dt.bfloat16
    AF = mybir.ActivationFunctionType
    OP = mybir.AluOpType

    nc = bacc.Bacc("TRN2", target_bir_lowering=False, debug=False)
    xx = nc.dram_tensor("xx", [128, 512], f32, kind="ExternalInput").ap()
    sout = nc.dram_tensor("sout", [80, 160], f32, kind="ExternalOutput").ap()

    with tile.TileContext(nc) as tc:
        with tc.tile_pool(name="cpool", bufs=1) as cpool, \
             tc.tile_pool(name="pool", bufs=2) as pool, \
             tc.tile_pool(name="ppool", bufs=2, space="PSUM") as ppool:
            scale_t = cpool.tile([128, 1], f32)
            bias_t = cpool.tile([128, 1], f32)
            if cfg["and_free"]:
                nc.vector.memset(scale_t[:], -2.0 * float(np.pi) / (1 << 16))
            else:
                nc.vector.memset(scale_t[:], 2.0 * float(np.pi) / (1 << 19))
            nc.vector.memset(bias_t[:], -float(np.pi))

            def body(_i=None):
                XX = pool.tile([128, 512], f32, tag="XX")
                U = pool.tile([128, 5120], f32, tag="U")
                C = pool.tile([128, 5120], bf16, tag="C")
                SO = pool.tile([128, 160], f32, tag="SO")
                Ui = U[:].bitcast(i32)
                Ui16 = U[:].bitcast(mybir.dt.int16)

                def cout_view(h, dd):
                    return C[:].rearrange(
                        "p (dd hh fg k b) -> p hh dd k fg b",
                        dd=2, hh=2, fg=16, b=8)[:, h, dd]

                # one load per half so downstream starts early
                for h in range(2):
                    deng = nc.gpsimd if (h == 1 and cfg["dma_split"]) else nc.sync
                    deng.dma_start(XX[:, 256 * h:256 * (h + 1)],
                                   xx[:, 256 * h:256 * (h + 1)])

                ps = [ppool.tile([80, 80], f32, name=f"ps{g}", tag=f"ps{g}")
                      for g in range(2)]

                for h in range(2):
                    # w = x*(k/2) + 16.75: low 19 mantissa bits hold the
                    # range-reduced phase (w's exponent is pinned to 4)
                    for k in range(KMAX):
                        teng = (nc.gpsimd if k in cfg["ts_pool_ks"]
                                else nc.vector)
                        teng.tensor_scalar(
                            U[:, h * 2560 + k * 256:h * 2560 + (k + 1) * 256],
                            XX[:, h * 256:(h + 1) * 256],
                            0.5 * k,
                            128.75 if cfg["and_free"] else 16.75,
                            OP.mult, OP.add)
                    if not cfg["and_free"]:
                        # m = bits(w) & 0x7FFFF  ->  (g + 0.5) * 2^19
                        nc.vector.tensor_scalar(
                            Ui[:, h * 2560:(h + 1) * 2560],
                            Ui[:, h * 2560:(h + 1) * 2560],
                            0x7FFFF, None, OP.bitwise_and)
                    for dd in range(2):
                        # c = sin(m*2pi/2^19 - pi) = cos(pi*k*x), bf16 out,
                        # scattered into (fg, k, b') so each matmul operand
                        # is one contiguous 80-column slice
                        if cfg["and_free"]:
                            # low int16 halves ARE the masked phase: the
                            # signed 16-bit wrap == mod 2^16, absorbed by
                            # sin periodicity (with a sign flip via the
                            # negative scale)
                            uin = Ui16.rearrange(
                                "p (hh k dd fg b t) -> p hh dd k fg b t",
                                hh=2, k=KMAX, dd=2, b=8, t=2)[:, h, dd, :, :, :, 0]
                            nc.scalar.activation(cout_view(h, dd), uin, AF.Sin,
                                                 bias=0.0, scale=scale_t[:])
                        else:
                            uin = Ui.rearrange(
                                "p (hh k dd fg b) -> p hh dd k fg b",
                                hh=2, k=KMAX, dd=2, b=8)[:, h, dd]
                            nc.scalar.activation(cout_view(h, dd), uin, AF.Sin,
                                                 bias=bias_t[:], scale=scale_t[:])

                    # matmuls for this half (both dims' C ready)
                    for fl in range(8):
                        for g in range(2):
                            fg = 16 * h + 2 * fl + g
                            nc.tensor.matmul(
                                ps[g][:],
                                C[:, fg * 80:fg * 80 + 80],
                                C[:, 2560 + fg * 80:2560 + fg * 80 + 80],
                                start=(h == 0 and fl == 0),
                                stop=(h == 1 and fl == 7))

                for g in range(2):
                    nc.scalar.copy(SO[0:80, g * 80:(g + 1) * 80], ps[g][:])
                nc.sync.dma_start(sout, SO[0:80, :])

            if loop:
                with tc.For_i(0, reps, 1) as i:
                    body(i)
            else:
                for _ in range(reps):
                    body()

    nc.compile()
    return nc


def _get_state():
    if "nc" not in _STATE:
        _STATE["nc"] = _build()
    return _STATE["nc"]


def _shard_inputs(x: np.ndarray):
    """x [512, 16, 32, 2] -> per-core {xx [128, 512]}.

    xx free layout: h*256 + d*128 + (fb - 128*h), i.e. per fg-half h the two
    coordinate planes side by side: [x0_h0 | x1_h0 | x0_h1 | x1_h1].
    """
    in_maps = []
    for c in range(NCORES):
        xc = x[c * TLOC:(c + 1) * TLOC]            # [64, 16, 32, 2]
        arr = xc.reshape(NF, 4, 16, 32, 2)         # (f, tp, b, a, d)
        arr = arr.transpose(4, 1, 3, 0, 2)         # (d, tp, a, f, b)
        arr = arr.reshape(2, 128, 256)             # p = tp*32+a, free = f*16+b
        xxc = np.concatenate([arr[0, :, :128], arr[1, :, :128],
                              arr[0, :, 128:], arr[1, :, 128:]], axis=1)
        in_maps.append({"xx": np.ascontiguousarray(xxc)})
    return in_maps


def _gather(souts):
    """souts: list of 8 [80, 160] partials -> scalar loss (float32).

    sout row m = k0*8 + b', col (80*g + k1*8 + b'') for batch b = 8*g + b'.
    """
    total = np.zeros((80, 160), dtype=np.float64)
    for s in souts:
        total += s.astype(np.float64)
    S = np.empty((B, KMAX, KMAX), dtype=np.float64)
    for g in range(2):
        for bp in range(8):
            S[8 * g + bp] = total[bp::8, 80 * g + bp:80 * (g + 1):8]
    nf, nw, cd = _np_constants()
    coeffs = S.reshape(B, KN) / (NA * T) / nf[None, :].astype(np.float64)
    d = nw[None, :].astype(np.float64) * (coeffs - cd[None, :].astype(np.float64))
    loss = np.mean(d * d)
    return np.float32(loss)


def kernel(x: np.ndarray) -> np.ndarray:
    from concourse.bass_utils import run_bass_kernel_spmd

    nc = _get_state()
    in_maps = _shard_inputs(np.asarray(x, dtype=np.float32))
    res = run_bass_kernel_spmd(nc, in_maps, list(range(NCORES)))
    souts = [r["sout"] for r in res.results]
    return _gather(souts)


# revision 35
# speedup vs baseline: 1.6846x; 1.6846x over previous
"""Trainium2 Bass kernel for the ergodicity loss.

Math: for x[T=512, B=16, N=32, d=2] in [0,1]^2 and modes (k0,k1) in {0..9}^2:
    basis = cos(pi*k0*x0) * cos(pi*k1*x1)                    (separable!)
    coeffs[b, k0, k1] = sum_{t,n} basis / (T*N) / nf[k1]
    loss = mean((nw * (coeffs - cd))**2)

Device strategy (8 cores, data-parallel over T: 64 timesteps/core):
  - Per core, per batch: 2048 points = 16 chunks x 128 partitions.
  - Both coordinate dims processed as ONE fused elementwise stream, in two
    fg-halves h for pipelining (xx[p, h*256 + dd*128 + fg_l*8 + b']):
      w = xx*(k/2) + 128.75     (DVE fused tensor_scalar per (k, h), 2x mode)
        w in [128.75, 133.25]: exponent pinned to 7, so the mantissa holds
        (v + 0.5)*2^16 in fixed point, where v = x*k/2 + 0.25 and
        sin(2*pi*v) == cos(pi*k*x).
      s = low int16 half of w    (no instruction at all -- the ACT input AP
        reads the low 16-bit lane of each fp32 at stride 2; the signed
        16-bit wraparound IS the mod-2^16 range reduction, absorbed by sin's
        periodicity up to a sign flip)
      c = Sin(s * (-2pi/2^16))   (ACT; arg in (-pi, pi] == Sin's valid table
        range; == -sin(2*pi*(v mod 1 -+ 0.5)) == sin(2*pi*v) == cos(pi*k*x)),
        written as bf16.
    Phase quantization at 2^-16 (~1e-4 rad) is far below the bf16 rounding
    of the cos values themselves; measured loss error vs reference ~7e-6.
  - coeffs partial sums on PE (bf16 inputs, fp32 PSUM): per chunk fg,
    lhsT = C0[:, fg*80:+80] (cols (k0, b')), rhs = C1 same -> PSUM[80, 80]
    accumulated over the 16 f-chunks; 2 batch-groups. Off-diagonal batch
    blocks are garbage, ignored at gather.
  - Output per core: sout[80, 160] raw PSUM dump (2 groups side by side).
Host: sum the 8 per-core partials, extract diagonal blocks, apply the tiny
[16, 100] normalization + weighted MSE.
"""
import numpy as np

T, B, NA, D = 512, 16, 32, 2
KMAX = 10
NCORES = 8
TLOC = T // NCORES          # 64 timesteps per core
NF = 16                     # point-chunks per batch (64*32/128)
KN = KMAX * KMAX

_STATE = {}

# tuning knobs: ts_pool_ks = k-values whose tensor_scalar runs on the (idle)
# Pool engine instead of DVE; dma_split = issue the h1 input DMA from the
# gpsimd engine so the two half-loads ride different DMA queues
CFG = {"ts_pool_ks": (), "dma_split": False, "and_free": True,
       "quarters": False}


def _np_constants():
    """Replicates reference._constants() exactly in numpy (L = ones)."""
    L = np.ones(D, dtype=np.float32)
    grids = np.meshgrid(*[np.arange(KMAX) for _ in range(D)], indexing="ij")
    K = np.stack(grids, -1).reshape(-1, D).astype(np.float32)          # [100, 2]
    k_scaled = K * np.pi / L
    nf = np.where(K[:, -1] != 0, np.sqrt(L[-1] / 2.0), 1.0).astype(np.float32)
    nw = ((1.0 + (k_scaled ** 2).sum(-1)) ** (-(D + 1) / 2.0) * 100.0).astype(np.float32)
    safe_k = np.where(K != 0, k_scaled, 1.0)
    term = np.where(K != 0,
                    (np.exp(1j * k_scaled * L) - 1.0) / (1j * safe_k * L),
                    1.0 + 0j)
    cd = (term.prod(-1).real / nf).astype(np.float32)                  # [100]
    return nf, nw, cd


def _build(reps: int = 1, loop: bool = False, cfg: dict | None = None):
    import concourse.tile as tile
    from concourse import bacc, mybir

    cfg = {**CFG, **(cfg or {})}
    f32 = mybir.dt.float32
    i32 = mybir.dt.int32
    bf16 = mybir.dt.bfloat16
    AF = mybir.ActivationFunctionType
    OP = mybir.AluOpType

    nc = bacc.Bacc("TRN2", target_bir_lowering=False, debug=False)
    xx = nc.dram_tensor("xx", [128, 512], f32, kind="ExternalInput").ap()
    sout = nc.dram_tensor("sout", [80, 160], f32, kind="ExternalOutput").ap()

    with tile.TileContext(nc) as tc:
        with tc.tile_pool(name="cpool", bufs=1) as cpool, \
             tc.tile_pool(name="pool", bufs=2) as pool, \
             tc.tile_pool(name="ppool", bufs=2, space="PSUM") as ppool:
            scale_t = cpool.tile([128, 1], f32)
            bias_t = cpool.tile([128, 1], f32)
            if cfg["and_free"]:
                nc.vector.memset(scale_t[:], -2.0 * float(np.pi) / (1 << 16))
            else:
                nc.vector.memset(scale_t[:], 2.0 * float(np.pi) / (1 << 19))
            nc.vector.memset(bias_t[:], -float(np.pi))

            def body(_i=None):
                XX = pool.tile([128, 512], f32, tag="XX")
                U = pool.tile([128, 5120], f32, tag="U")
                C = pool.tile([128, 5120], bf16, tag="C")
                SO = pool.tile([128, 160], f32, tag="SO")
                Ui = U[:].bitcast(i32)
                Ui16 = U[:].bitcast(mybir.dt.int16)

                def cout_view(h, dd):
                    return C[:].rearrange(
                        "p (dd hh fg k b) -> p hh dd k fg b",
                        dd=2, hh=2, fg=16, b=8)[:, h, dd]

                # one load per chunk so downstream starts early
                nld = 4 if cfg["quarters"] else 2
                w_ld = 512 // nld
                for q in range(nld):
                    deng = nc.gpsimd if (q >= nld // 2 and cfg["dma_split"]) \
                        else nc.sync
                    deng.dma_start(XX[:, w_ld * q:w_ld * (q + 1)],
                                   xx[:, w_ld * q:w_ld * (q + 1)])

                ps = [ppool.tile([80, 80], f32, name=f"ps{g}", tag=f"ps{g}")
                      for g in range(2)]

                for h in range(2):
                    # w = x*(k/2) + C: the mantissa of w holds the
                    # range-reduced phase in fixed point (exponent pinned).
                    # quarters=True emits per-(k, dim) [128, 128] TS so each
                    # sin only waits for its own quarter's chain.
                    dd_splits = ((0, 1),) if not cfg["quarters"] else ((0,), (1,))
                    for dds in dd_splits:
                        for k in range(KMAX):
                            teng = (nc.gpsimd if k in cfg["ts_pool_ks"]
                                    else nc.vector)
                            o = h * 2560 + k * 256 + dds[0] * 128
                            wdt = 128 * len(dds)
                            teng.tensor_scalar(
                                U[:, o:o + wdt],
                                XX[:, h * 256 + dds[0] * 128:
                                   h * 256 + dds[0] * 128 + wdt],
                                0.5 * k,
                                128.75 if cfg["and_free"] else 16.75,
                                OP.mult, OP.add)
                    if not cfg["and_free"]:
                        # m = bits(w) & 0x7FFFF  ->  (g + 0.5) * 2^19
                        nc.vector.tensor_scalar(
                            Ui[:, h * 2560:(h + 1) * 2560],
                            Ui[:, h * 2560:(h + 1) * 2560],
                            0x7FFFF, None, OP.bitwise_and)
                    for dd in range(2):
                        # c = sin(m*2pi/2^19 - pi) = cos(pi*k*x), bf16 out,
                        # scattered into (fg, k, b') so each matmul operand
                        # is one contiguous 80-column slice
                        if cfg["and_free"]:
                            # low int16 halves ARE the masked phase: the
                            # signed 16-bit wrap == mod 2^16, absorbed by
                            # sin periodicity (with a sign flip via the
                            # negative scale)
                            uin = Ui16.rearrange(
                                "p (hh k dd fg b t) -> p hh dd k fg b t",
                                hh=2, k=KMAX, dd=2, b=8, t=2)[:, h, dd, :, :, :, 0]
                            nc.scalar.activation(cout_view(h, dd), uin, AF.Sin,
                                                 bias=0.0, scale=scale_t[:])
                        else:
                            uin = Ui.rearrange(
                                "p (hh k dd fg b) -> p hh dd k fg b",
                                hh=2, k=KMAX, dd=2, b=8)[:, h, dd]
                            nc.scalar.activation(cout_view(h, dd), uin, AF.Sin,
                                                 bias=bias_t[:], scale=scale_t[:])

                    # matmuls for this half (both dims' C ready)
                    for fl in range(8):
                        for g in range(2):
                            fg = 16 * h + 2 * fl + g
                            nc.tensor.matmul(
                                ps[g][:],
                                C[:, fg * 80:fg * 80 + 80],
                                C[:, 2560 + fg * 80:2560 + fg * 80 + 80],
                                start=(h == 0 and fl == 0),
                                stop=(h == 1 and fl == 7))

                for g in range(2):
                    nc.scalar.copy(SO[0:80, g * 80:(g + 1) * 80], ps[g][:])
                nc.sync.dma_start(sout, SO[0:80, :])

            if loop:
                with tc.For_i(0, reps, 1) as i:
                    body(i)
            else:
                for _ in range(reps):
                    body()

    nc.compile()
    return nc


def _get_state():
    if "nc" not in _STATE:
        _STATE["nc"] = _build()
    return _STATE["nc"]


def _shard_inputs(x: np.ndarray):
    """x [512, 16, 32, 2] -> per-core {xx [128, 512]}.

    xx free layout: h*256 + d*128 + (fb - 128*h), i.e. per fg-half h the two
    coordinate planes side by side: [x0_h0 | x1_h0 | x0_h1 | x1_h1].
    """
    in_maps = []
    for c in range(NCORES):
        xc = x[c * TLOC:(c + 1) * TLOC]            # [64, 16, 32, 2]
        arr = xc.reshape(NF, 4, 16, 32, 2)         # (f, tp, b, a, d)
        arr = arr.transpose(4, 1, 3, 0, 2)         # (d, tp, a, f, b)
        arr = arr.reshape(2, 128, 256)             # p = tp*32+a, free = f*16+b
        xxc = np.concatenate([arr[0, :, :128], arr[1, :, :128],
                              arr[0, :, 128:], arr[1, :, 128:]], axis=1)
        in_maps.append({"xx": np.ascontiguousarray(xxc)})
    return in_maps


def _gather(souts):
    """souts: list of 8 [80, 160] partials -> scalar loss (float32).

    sout row m = k0*8 + b', col (80*g + k1*8 + b'') for batch b = 8*g + b'.
    """
    total = np.zeros((80, 160), dtype=np.float64)
    for s in souts:
        total += s.astype(np.float64)
    S = np.empty((B, KMAX, KMAX), dtype=np.float64)
    for g in range(2):
        for bp in range(8):
            S[8 * g + bp] = total[bp::8, 80 * g + bp:80 * (g + 1):8]
    nf, nw, cd = _np_constants()
    coeffs = S.reshape(B, KN) / (NA * T) / nf[None, :].astype(np.float64)
    d = nw[None, :].astype(np.float64) * (coeffs - cd[None, :].astype(np.float64))
    loss = np.mean(d * d)
    return np.float32(loss)


def kernel(x: np.ndarray) -> np.ndarray:
    from concourse.bass_utils import run_bass_kernel_spmd

    nc = _get_state()
    in_maps = _shard_inputs(np.asarray(x, dtype=np.float32))
    res = run_bass_kernel_spmd(nc, in_maps, list(range(NCORES)))
    souts = [r["sout"] for r in res.results]
    return _gather(souts)
